# revision 7
# baseline (speedup 1.0000x reference)
"""Segment-mean (graph pooling) kernel for Trainium2, 8 NeuronCores.

reference semantics:
    sums   = segment_sum(node_h, node_batch, num_segments=G)
    counts = segment_sum(ones(N), node_batch, G)
    out    = sums / max(counts, 1)[:, None]

node_batch is sorted, so segments are contiguous row runs. Core c owns
segments [128c, 128(c+1)) and reads exactly those segments' rows, cast
to bf16 on the host (input quantization ~2e-3 rel err, well under the
2e-2 gate) and zero-padded per segment to a multiple of 128 rows. With
that padding every 128-row tile belongs to exactly one segment, so the
device only needs per-tile column sums plus a tiny tile->segment
routing matmul:

  stage 1: per tile t, tilesum_t = ones(128) @ rows_t via PE matmuls
           whose lhsT is a shifted one-hot-column "staircase" slice
           (M=32 so out.base_partition stays in {0,32,64,96}), so 32
           consecutive tiles accumulate into one PSUM [32,128] block;
           a chunk of 128 tiles fills a PSUM [128,128] tile.
  stage 2: cast chunk tilesums to bf16 (DVE) and matmul with a
           DVE-built one-hot [tile, seg] selector, accumulating
           [128 segs, 128 feat] in PSUM across all chunks.

Epilogue scales by 1/max(count,1). DMA is the bottleneck by design:
~67 MB/core of bf16 at the ~358 GB/s HBM/NC limit, while PE does one
128-col matmul per tile and DVE does almost nothing.
"""

import os

import numpy as np
import ml_dtypes

BF16 = ml_dtypes.bfloat16
P = 128  # partitions / rows per tile
D = 128  # feature dim
G = 1024  # num segments
N_CORES = 8
SLAB = 64  # node-tiles per DMA slab (2 MiB per slab)
CHUNK = 128  # tiles per PSUM chunk (= 2 slabs)
SLAB_BUFS = 6
SENTINEL = 200.0  # tileseg id outside [0,128) -> routed nowhere

_prog_cache: dict[int, object] = {}
LAST_RESULT = None  # BassKernelResults of the most recent device run


def _np_fallback(node_h, node_batch, num_graphs):
    node_h = np.asarray(node_h, dtype=np.float32)
    nb = np.asarray(node_batch).astype(np.int64)
    ng = int(num_graphs)
    sums = np.zeros((ng, node_h.shape[1]), dtype=np.float32)
    np.add.at(sums, nb, node_h)
    counts = np.bincount(nb, minlength=ng).astype(np.float32)
    return sums / np.maximum(counts, 1.0)[:, None]


def _build_program(T: int):
    import concourse.bacc as bacc
    import concourse.mybir as mybir
    import concourse.tile as tile

    bf16 = mybir.dt.bfloat16
    f32 = mybir.dt.float32

    assert T % CHUNK == 0
    n_chunks = T // CHUNK
    n_slabs = T // SLAB
    META_W = P + 127 + n_chunks  # iota | staircase | tileseg

    nc = bacc.Bacc(None)
    h_in = nc.dram_tensor("h", [P, T * D], bf16, kind="ExternalInput")
    meta_in = nc.dram_tensor("meta", [P, META_W], bf16, kind="ExternalInput")
    recip_in = nc.dram_tensor("recip", [P, 1], f32, kind="ExternalInput")
    out_t = nc.dram_tensor("out", [P, D], f32, kind="ExternalOutput")

    with tile.TileContext(nc) as tc:
        with (
            tc.tile_pool(name="const", bufs=1) as constp,
            tc.tile_pool(name="slabs", bufs=SLAB_BUFS) as slabp,
            tc.tile_pool(name="ts", bufs=2) as tsp,
            tc.tile_pool(name="oh", bufs=2) as ohp,
            tc.tile_pool(name="chunk", bufs=2, space="PSUM") as chunkp,
            tc.tile_pool(name="acc", bufs=1, space="PSUM") as accp,
            tc.tile_pool(name="outp", bufs=1) as outp,
        ):
            meta_sb = constp.tile([P, META_W], bf16)
            nc.sync.dma_start(meta_sb[:], meta_in[:])
            recip_sb = constp.tile([P, 1], f32)
            nc.sync.dma_start(recip_sb[:], recip_in[:])
            iota = meta_sb[:, 0:P]
            stair = meta_sb[:, P : P + 127]
            tileseg0 = P + 127

            acc = accp.tile([P, D], f32)

            slabs = {}

            for k in range(n_chunks):
                # tile->segment one-hot for this chunk's 128 tiles
                oh = ohp.tile([P, P], bf16)
                nc.vector.tensor_tensor(
                    out=oh[:],
                    in0=iota,
                    in1=meta_sb[:, tileseg0 + k : tileseg0 + k + 1].to_broadcast(
                        [P, P]
                    ),
                    op=mybir.AluOpType.is_equal,
                )

                # stage 1: column sums of 128 tiles into PSUM rows
                cp = chunkp.tile([P, D], f32)
                for a in range(2):
                    for j in range(64):
                        t = k * CHUNK + a * 64 + j
                        g = t // SLAB
                        if t % SLAB == 0:
                            slabs[g] = slabp.tile(
                                [P, SLAB * D], bf16, name="slab"
                            )
                            nc.gpsimd.dma_start(
                                slabs[g][:],
                                h_in[:, g * SLAB * D : (g + 1) * SLAB * D],
                            )
                        pos = t % SLAB
                        nc.tensor.matmul(
                            out=cp[a * 64 : (a + 1) * 64, :],
                            lhsT=stair[:, 63 - j : 127 - j],
                            rhs=slabs[g][:, pos * D : (pos + 1) * D],
                            start=(j == 0),
                            stop=(j == 63),
                        )

                # stage 2: route tilesums to segment rows
                ts = tsp.tile([P, D], bf16)
                nc.vector.tensor_copy(out=ts[:], in_=cp[:])
                nc.tensor.matmul(
                    out=acc[:],
                    lhsT=oh[:],
                    rhs=ts[:],
                    start=(k == 0),
                    stop=(k == n_chunks - 1),
                    skip_group_check=True,
                )

            res = outp.tile([P, D], f32)
            nc.vector.tensor_tensor(
                out=res[:],
                in0=acc[:],
                in1=recip_sb[:, 0:1].to_broadcast([P, D]),
                op=mybir.AluOpType.mult,
            )
            nc.sync.dma_start(out_t[:], res[:])

    nc.finalize()
    return nc


def kernel(node_h, node_batch, num_graphs):
    global LAST_RESULT
    node_h = np.asarray(node_h)
    nb = np.asarray(node_batch)
    ng = int(num_graphs)

    N = node_h.shape[0]
    if (
        ng != G
        or node_h.ndim != 2
        or node_h.shape[1] != D
        or nb.shape != (N,)
        or np.any(nb[:-1] > nb[1:])
        or nb[0] < 0
        or nb[-1] >= G
    ):
        return _np_fallback(node_h, node_batch, num_graphs)

    node_h = np.ascontiguousarray(node_h, dtype=np.float32)
    nb = nb.astype(np.int64)

    seg_per_core = G // N_CORES
    counts = np.bincount(nb, minlength=G)
    bounds = np.concatenate([[0], np.cumsum(counts)])
    kt = -(-counts // P)  # tiles per segment after zero padding
    per_core_tiles = kt.reshape(N_CORES, seg_per_core).sum(axis=1)
    T = int(-(-per_core_tiles.max() // CHUNK) * CHUNK)
    if T < 2 * SLAB or T > 4096:
        return _np_fallback(node_h, node_batch, num_graphs)
    n_chunks = T // CHUNK

    iota = np.tile(np.arange(P, dtype=np.float32), (P, 1))
    stair = np.zeros((P, 127), dtype=np.float32)
    stair[:, 63] = 1.0

    in_maps = []
    for c in range(N_CORES):
        s0 = c * seg_per_core
        r0, r1 = int(bounds[s0]), int(bounds[s0 + seg_per_core])
        blk = node_h[r0:r1].astype(BF16)

        h = np.zeros((P, T * D), dtype=BF16)
        ts_flat = np.full(T, SENTINEL, dtype=np.float32)
        off = 0
        for i in range(seg_per_core):
            s = s0 + i
            cnt = int(counts[s])
            if cnt == 0:
                continue
            k = int(kt[s])
            stage = np.zeros((P * k, D), dtype=BF16)
            stage[:cnt] = blk[bounds[s] - r0 : bounds[s + 1] - r0]
            h[:, off * D : (off + k) * D] = stage.reshape(P, k * D)
            ts_flat[off : off + k] = i
            off += k

        tileseg = np.ascontiguousarray(ts_flat.reshape(n_chunks, P).T)
        meta = np.concatenate([iota, stair, tileseg], axis=1).astype(BF16)
        recip = (
            1.0
            / np.maximum(counts[s0 : s0 + seg_per_core], 1.0).astype(np.float32)
        ).reshape(P, 1)

        in_maps.append({"h": h, "meta": meta, "recip": recip})

    if T not in _prog_cache:
        _prog_cache[T] = _build_program(T)
    nc = _prog_cache[T]

    from concourse.bass_utils import run_bass_kernel_spmd

    trace = bool(os.environ.get("KERNEL_TRACE"))
    result = run_bass_kernel_spmd(
        nc,
        in_maps,
        core_ids=list(range(N_CORES)),
        trace=trace,
        trace_cores=list(range(N_CORES)) if trace else None,
    )
    LAST_RESULT = result

    out = np.concatenate([result.results[c]["out"] for c in range(N_CORES)], axis=0)
    return out.astype(np.float32)
